# revision 14
# baseline (speedup 1.0000x reference)
"""Trainium2 Bass kernel for batched DWT (db4, single level) via banded matmul.

Problem: x [1024, 4096] f32, W [4096, 4096] f32 wavelet analysis matrix
(transposed banded circulant built from the 8-tap db4 filter pair).
    y = x @ W;  out = concat([y[:, ::2], y[:, 1::2]], axis=1)

Key structure: W[j, n] is nonzero only for j - 2*(n//2) in [0, 8) (mod 4096).
So output columns [122*i, 122*i+122) depend only on x columns
[122*i, 122*i+128) (mod 4096), and the 128x122 coefficient block is the SAME
for every i (circulant shift invariance). Instead of a dense 4096x4096 matmul
(64 MB of W traffic per core) each core does 34 small PE matmuls against one
shared 128x122 band matrix extracted from W's top-left corner, with the
even/odd de-interleave folded into the band matrix's column order.

Sharding: pure data parallel over batch. Each of the 8 cores gets 128 rows.
The host pre-transposes its shard into the lhsT (stationary operand) tile
layout H[:, 128i:128i+128] = x_shard.T[122i : 122i+128, :] (circular pad),
with the band matrix prepended as the first 122 columns so the whole working
set arrives in 4 chunked DMAs (~4.3 MB HBM traffic per core, memory-bound).
NB: every matmul is constructed to need at most ONE semaphore wait — walrus
cannot encode more than one on the fp32 LDWEIGHTS+MM pair.
"""

import numpy as np

import concourse.bacc as bacc
import concourse.tile as tile
from concourse import mybir
from concourse.bass_utils import run_bass_kernel_spmd

N_CORES = 8
BATCH = 1024
SEQ = 4096
R = BATCH // N_CORES          # rows per core = 128
P = 128                       # partitions
BLK = 122                     # output columns per block (122 + 6 tap halo = 128)
NBLK = 34                     # ceil(4096 / 122); last block has 70 real columns
HALF = BLK // 2               # 61 even (approx) + 61 odd (detail) cols per block
HCOLS = BLK + NBLK * P        # 122 (band matrix) + 4352 (lhsT tiles)

# chunks of blocks: (first block, n blocks). Each chunk = one input DMA,
# one output DMA; psum groups of <=4 blocks inside.
CHUNKS = [(0, 8), (8, 8), (16, 8), (24, 10)]

FP32 = mybir.dt.float32

# tuning knobs (see _build_bass); defaults picked via TimelineSim + HW slope
OPTS = {
    "chunks": CHUNKS,
    "alt_copy": True,    # alternate deinterleave copies between DVE and ACT
    "alt_load": True,    # alternate load DMAs between the two HWDGE rings
    "mm_dtype": "f32",   # "f32" | "f32r" (bitcast matmul operands to float32r)
}

_CACHE = {}


def _build_bass(repeat=1, opts=None):
    """Build (once) the single-core Bass/Tile program; all 8 cores run it SPMD.

    repeat > 1 replicates the whole body back-to-back inside one NEFF —
    used only for benchmarking (wall-clock slope vs repeat count isolates
    per-pass HW time from host/tunnel dispatch overhead)."""
    o = dict(OPTS, **(opts or {}))
    chunks = o["chunks"]
    nc = bacc.Bacc(
        "TRN2",
        target_bir_lowering=False,
        debug=False,
        enable_asserts=False,
        num_devices=N_CORES,
    )
    h_t = nc.dram_tensor("h", [P, HCOLS], FP32, kind="ExternalInput")
    out_t = nc.dram_tensor("out", [R, SEQ], FP32, kind="ExternalOutput")
    h_ap = h_t.ap()
    out_ap = out_t.ap()

    with tile.TileContext(nc) as tc:
        with (
            tc.tile_pool(name="hpool", bufs=4) as hp,
            tc.tile_pool(name="opool", bufs=4) as op,
            tc.tile_pool(name="psum", bufs=8, space="PSUM") as psump,
        ):
            # out DRAM viewed as [p, 2 halves, 2048]: half 0 = approx, 1 = detail
            out_v = out_ap.rearrange("p (s m) -> p s m", s=2)

            btile = None
            for b0, nb in [c for _ in range(repeat) for c in CHUNKS]:
                # chunk 0's DMA also carries the 122-col band matrix so the
                # first matmuls need exactly one DMA wait.
                lead = BLK if b0 == 0 else 0
                dcol0 = BLK + P * b0 - lead
                ht = hp.tile([P, lead + P * nb], FP32, tag="h")
                nc.sync.dma_start(ht[:], h_ap[:, dcol0 : BLK + P * (b0 + nb)])
                if b0 == 0:
                    btile = ht  # band matrix lives in cols [0:122] of chunk 0

                # number of real output cols this chunk contributes per half
                ceff = min(HALF * (b0 + nb), SEQ // 2) - HALF * b0
                otile = op.tile([P, 2 * ceff], FP32, tag="o")
                o_v = otile[:].rearrange("p (s m) -> p s m", s=2)

                for g0 in range(0, nb, 4):
                    gn = min(4, nb - g0)
                    ps = psump.tile([P, BLK * 4], FP32, tag="ps")
                    for q in range(gn):
                        blk = b0 + g0 + q
                        col = lead + P * (blk - b0) if b0 == 0 else P * (blk - b0)
                        nc.tensor.matmul(
                            ps[:, BLK * q : BLK * (q + 1)],
                            ht[:, col : col + P],
                            btile[:, 0:BLK],
                            start=True,
                            stop=True,
                        )
                    # de-interleaving PSUM -> SBUF copy. Full blocks in one
                    # 4D-AP copy; the final 70-wide block separately.
                    nfull = gn if b0 + g0 + gn < NBLK else gn - 1
                    loc0 = HALF * g0  # chunk-local col offset of group
                    if nfull:
                        src = ps[:, 0 : BLK * nfull].rearrange(
                            "p (g s t) -> p g s t", s=2, t=HALF
                        )
                        dst = o_v[:, :, loc0 : loc0 + HALF * nfull].rearrange(
                            "p s (g t) -> p g s t", t=HALF
                        )
                        nc.vector.tensor_copy(dst, src)
                    if nfull != gn:  # last block: 70 real cols = 35 + 35
                        src = ps[:, BLK * nfull : BLK * (nfull + 1)].rearrange(
                            "p (s t) -> p s t", t=HALF
                        )[:, :, 0:35]
                        dst = o_v[:, :, loc0 + HALF * nfull : loc0 + HALF * nfull + 35]
                        nc.vector.tensor_copy(dst, src)

                nc.scalar.dma_start(
                    out_v[:, :, HALF * b0 : HALF * b0 + ceff], o_v[:, :, :]
                )

    _strip_redundant_matmul_self_waits(nc)
    nc.compile()
    return nc


def _strip_redundant_matmul_self_waits(nc):
    """Walrus can encode only one sync wait on fp32 LDWEIGHTS+MM pairs and on
    DVE 4D-AP TensorCopy encodings. Tile emits per-proc waits without
    transitive reduction, so e.g. a psum slot-reuse matmul carries a wait on
    the PE's OWN semaphore (covering the slot's previous writers). Engines
    execute their queues in FIFO order and these semaphores only ever
    increment, so a wait on engine E's own semaphore whose value is already
    covered by E's earlier increments is trivially satisfied by program
    order — drop it (only when the instruction exceeds one wait)."""
    for f in nc.m.functions:
        cum = {}  # (engine, sem_name) -> increments issued earlier on engine
        for b in f.blocks:
            for inst in b.instructions:
                eng = getattr(inst, "engine", None)
                si = inst.sync_info
                if si is None:
                    continue
                if len(si.on_wait) > 1 and type(inst).__name__ != "InstDrain":
                    keep = [
                        w
                        for w in si.on_wait
                        if cum.get((eng, w.ant_name), 0) < w.wait_value
                    ]
                    if len(keep) != len(si.on_wait):
                        si.on_wait = keep
                        inst.sync_info = si
                for u in si.on_update:
                    k = (eng, u.ant_name)
                    cum[k] = cum.get(k, 0) + u.update_value


def _get_nc(repeat=1):
    key = ("nc", repeat)
    if key not in _CACHE:
        _CACHE[key] = _build_bass(repeat)
    return _CACHE[key]


def _pack_host(x, bmat):
    """Per-core input tensors: [band matrix | lhsT tiles], where lhsT tile i
    is x_shard.T[122i : 122i+128, :] (circularly padded)."""
    hs = []
    for c in range(N_CORES):
        xs = np.ascontiguousarray(x[R * c : R * (c + 1)].T)  # [4096, 128]
        xtp = np.concatenate([xs, xs[:P]], axis=0)            # circular pad
        H = np.empty((P, HCOLS), dtype=np.float32)
        H[:, 0:BLK] = bmat
        for i in range(NBLK):
            H[:, BLK + P * i : BLK + P * (i + 1)] = xtp[BLK * i : BLK * i + P]
        hs.append(H)
    return hs


def _band_matrix(W):
    """128x122 coefficient block with de-interleaved (evens-first) columns."""
    perm = np.concatenate([np.arange(0, BLK, 2), np.arange(1, BLK, 2)])
    return np.ascontiguousarray(np.asarray(W, dtype=np.float32)[0:P, perm])


def run(x, W, trace=False):
    x = np.ascontiguousarray(np.asarray(x, dtype=np.float32))
    assert x.shape == (BATCH, SEQ), x.shape
    in_maps = [{"h": h} for h in _pack_host(x, _band_matrix(W))]
    res = run_bass_kernel_spmd(
        _get_nc(), in_maps, core_ids=list(range(N_CORES)), trace=trace
    )
    out = np.concatenate([res.results[c]["out"] for c in range(N_CORES)], axis=0)
    return out, res


def kernel(x, W):
    out, _ = run(x, W)
    return out


# revision 27
# speedup vs baseline: 4.6253x; 4.6253x over previous
"""Trainium2 Bass kernel for batched DWT (db4, single level) via banded matmul.

Problem: x [1024, 4096] f32, W [4096, 4096] f32 wavelet analysis matrix
(transposed banded circulant built from the 8-tap db4 filter pair).
    y = x @ W;  out = concat([y[:, ::2], y[:, 1::2]], axis=1)

Key structure: W[j, n] is nonzero only for j - 2*(n//2) in [0, 8) (mod 4096).
So output columns [122*i, 122*i+122) depend only on x columns
[122*i, 122*i+128) (mod 4096), and the 128x122 coefficient block is the SAME
for every i (circulant shift invariance). Instead of a dense 4096x4096 matmul
(64 MB of W traffic per core) each core does 34 small PE matmuls against one
shared 128x122 band matrix extracted from W's top-left corner, with the
even/odd de-interleave folded into the band matrix's column order.

Sharding: pure data parallel over batch. Each of the 8 cores gets 128 rows.
The host pre-transposes its shard into the lhsT (stationary operand) tile
layout H[:, 128i:128i+128] = x_shard.T[122i : 122i+128, :] (circular pad),
with the band matrix prepended as the first 122 columns so the whole working
set arrives in a few chunked DMAs (~4.3 MB HBM traffic per core, memory-bound:
~12 us of DMA at ~360 GB/s/core vs ~7 us of PE work hidden under it).
"""

import numpy as np

import concourse.bacc as bacc
import concourse.tile as tile
from concourse import mybir
from concourse.bass_utils import run_bass_kernel_spmd

N_CORES = 8
BATCH = 1024
SEQ = 4096
R = BATCH // N_CORES          # rows per core = 128
P = 128                       # partitions
BLK = 122                     # output columns per block (122 + 6 tap halo = 128)
NBLK = 34                     # ceil(4096 / 122); last block has 70 real columns
HALF = BLK // 2               # 61 even (approx) + 61 odd (detail) cols per block
HCOLS = BLK + NBLK * P        # 122 (band matrix) + 4352 (lhsT tiles)

# chunks of blocks: (first block, n blocks). Each chunk = one input DMA,
# one output DMA; psum groups of <=4 blocks inside. Progressive sizes: small
# first chunk -> PE starts early; small last chunk -> short exposed tail store.
CHUNKS = [(0, 1), (1, 4), (5, 9), (14, 10), (24, 7), (31, 3)]

FP32 = mybir.dt.float32

# tuning knobs (see _build_bass); defaults picked via TimelineSim + HW slope
OPTS = {
    "chunks": CHUNKS,
    "alt_copy": True,    # alternate deinterleave copies between DVE and ACT
    "alt_load": True,    # alternate load DMAs between the two HWDGE rings
    "mm_dtype": "f32",   # "f32" | "f32r" (bitcast matmul operands to float32r)
}

_CACHE = {}


def _build_bass(repeat=1, opts=None):
    """Build (once) the single-core Bass/Tile program; all 8 cores run it SPMD.

    repeat > 1 replicates the whole body back-to-back inside one NEFF —
    used only for benchmarking (wall-clock slope vs repeat count isolates
    per-pass HW time from host/tunnel dispatch overhead)."""
    o = dict(OPTS, **(opts or {}))
    chunks = o["chunks"]
    loop_n = o.get("loop_n", 0)  # >0: wrap body in a HW loop (bench only)
    nc = bacc.Bacc(
        "TRN2",
        target_bir_lowering=False,
        debug=False,
        enable_asserts=False,
        num_devices=N_CORES,
    )
    h_t = nc.dram_tensor("h", [P, HCOLS], FP32, kind="ExternalInput")
    out_t = nc.dram_tensor("out", [R, SEQ], FP32, kind="ExternalOutput")
    h_ap = h_t.ap()
    out_ap = out_t.ap()

    with tile.TileContext(nc) as tc:
        with (
            tc.tile_pool(name="hpool", bufs=4) as hp,
            tc.tile_pool(name="opool", bufs=4) as op,
            tc.tile_pool(name="psum", bufs=8, space="PSUM") as psump,
        ):
            # out DRAM viewed as [p, 2 halves, 2048]: half 0 = approx, 1 = detail
            out_v = out_ap.rearrange("p (s m) -> p s m", s=2)

            def mm_ap(ap):
                if o["mm_dtype"] == "f32r":
                    return ap.bitcast(mybir.dt.float32r)
                return ap

            def emit_pass():
                btile = None
                copy_i = 0
                for ci, (b0, nb) in enumerate(chunks):
                    btile, copy_i = emit_chunk(ci, b0, nb, btile, copy_i)

            def emit_chunk(ci, b0, nb, btile, copy_i):
                # chunk 0's DMA also carries the 122-col band matrix so the
                # first matmuls need exactly one DMA wait.
                lead = BLK if b0 == 0 else 0
                dcol0 = BLK + P * b0 - lead
                ht = hp.tile([P, lead + P * nb], FP32, tag="h")
                ld_eng = nc.scalar if (o["alt_load"] and ci % 2) else nc.sync
                ld_eng.dma_start(ht[:], h_ap[:, dcol0 : BLK + P * (b0 + nb)])
                if b0 == 0:
                    btile = ht  # band matrix lives in cols [0:122] of chunk 0

                # number of real output cols this chunk contributes per half
                ceff = min(HALF * (b0 + nb), SEQ // 2) - HALF * b0
                otile = op.tile([P, 2 * ceff], FP32, tag="o")
                o_v = otile[:].rearrange("p (s m) -> p s m", s=2)

                def copy(dst, src):
                    nonlocal copy_i
                    if o["alt_copy"] and copy_i % 2:
                        nc.scalar.copy(dst, src)
                    else:
                        nc.vector.tensor_copy(dst, src)
                    copy_i += 1

                for g0 in range(0, nb, 4):
                    gn = min(4, nb - g0)
                    ps = psump.tile([P, BLK * 4], FP32, tag="ps")
                    for q in range(gn):
                        blk = b0 + g0 + q
                        col = lead + P * (blk - b0) if b0 == 0 else P * (blk - b0)
                        nc.tensor.matmul(
                            ps[:, BLK * q : BLK * (q + 1)],
                            mm_ap(ht[:, col : col + P]),
                            mm_ap(btile[:, 0:BLK]),
                            start=True,
                            stop=True,
                        )
                    # de-interleaving PSUM -> SBUF copy. Full blocks in one
                    # 4D-AP copy; the final 70-wide block separately.
                    nfull = gn if (b0 + g0 + gn) % NBLK else gn - 1
                    loc0 = HALF * g0  # chunk-local col offset of group
                    if nfull:
                        src = ps[:, 0 : BLK * nfull].rearrange(
                            "p (g s t) -> p g s t", s=2, t=HALF
                        )
                        dst = o_v[:, :, loc0 : loc0 + HALF * nfull].rearrange(
                            "p s (g t) -> p g s t", t=HALF
                        )
                        copy(dst, src)
                    if nfull != gn:  # last block: 70 real cols = 35 + 35
                        src = ps[:, BLK * nfull : BLK * (nfull + 1)].rearrange(
                            "p (s t) -> p s t", t=HALF
                        )[:, :, 0:35]
                        dst = o_v[:, :, loc0 + HALF * nfull : loc0 + HALF * nfull + 35]
                        copy(dst, src)

                st_eng = nc.sync if (o["alt_load"] and ci % 2) else nc.scalar
                st_eng.dma_start(
                    out_v[:, :, HALF * b0 : HALF * b0 + ceff], o_v[:, :, :]
                )
                return btile, copy_i

            if loop_n:
                with tc.For_i(0, loop_n, 1):
                    emit_pass()
            else:
                for _ in range(repeat):
                    emit_pass()

    # Note: instructions that end up with >1 sync wait (walrus encodes only
    # one on fp32 LDW+MM pairs etc.) are legalized by bacc's compile() below.
    nc.compile()
    return nc


def _get_nc(repeat=1, opts=None):
    key = ("nc", repeat, repr(sorted((opts or {}).items(), key=str)))
    if key not in _CACHE:
        _CACHE[key] = _build_bass(repeat, opts)
    return _CACHE[key]


def _pack_host(x, bmat):
    """Per-core input tensors: [band matrix | lhsT tiles], where lhsT tile i
    is x_shard.T[122i : 122i+128, :] (circularly padded)."""
    hs = []
    for c in range(N_CORES):
        xs = np.ascontiguousarray(x[R * c : R * (c + 1)].T)  # [4096, 128]
        xtp = np.concatenate([xs, xs[:P]], axis=0)            # circular pad
        H = np.empty((P, HCOLS), dtype=np.float32)
        H[:, 0:BLK] = bmat
        for i in range(NBLK):
            H[:, BLK + P * i : BLK + P * (i + 1)] = xtp[BLK * i : BLK * i + P]
        hs.append(H)
    return hs


def _band_matrix(W):
    """128x122 coefficient block with de-interleaved (evens-first) columns."""
    perm = np.concatenate([np.arange(0, BLK, 2), np.arange(1, BLK, 2)])
    return np.ascontiguousarray(np.asarray(W, dtype=np.float32)[0:P, perm])


def run(x, W, trace=False):
    x = np.ascontiguousarray(np.asarray(x, dtype=np.float32))
    assert x.shape == (BATCH, SEQ), x.shape
    in_maps = [{"h": h} for h in _pack_host(x, _band_matrix(W))]
    res = run_bass_kernel_spmd(
        _get_nc(), in_maps, core_ids=list(range(N_CORES)), trace=trace
    )
    out = np.concatenate([res.results[c]["out"] for c in range(N_CORES)], axis=0)
    return out, res


def kernel(x, W):
    out, _ = run(x, W)
    return out


# revision 31
# speedup vs baseline: 4.6956x; 1.0152x over previous
"""Trainium2 Bass kernel for batched DWT (db4, single level) via banded matmul.

Problem: x [1024, 4096] f32, W [4096, 4096] f32 wavelet analysis matrix
(transposed banded circulant built from the 8-tap db4 filter pair).
    y = x @ W;  out = concat([y[:, ::2], y[:, 1::2]], axis=1)

Key structure: W[j, n] is nonzero only for j - 2*(n//2) in [0, 8) (mod 4096).
So output columns [122*i, 122*i+122) depend only on x columns
[122*i, 122*i+128) (mod 4096), and the 128x122 coefficient block is the SAME
for every i (circulant shift invariance). Instead of a dense 4096x4096 matmul
(64 MB of W traffic per core) each core does 34 small PE matmuls against one
shared 128x122 band matrix extracted from W's top-left corner, with the
even/odd de-interleave folded into the band matrix's column order.

Sharding: pure data parallel over batch. Each of the 8 cores gets 128 rows.
The host pre-transposes its shard into the lhsT (stationary operand) tile
layout H[:, 128i:128i+128] = x_shard.T[122i : 122i+128, :] (circular pad),
with the band matrix prepended as the first 122 columns so the whole working
set arrives in a few chunked DMAs (~4.3 MB HBM traffic per core, memory-bound:
~12 us of DMA at ~360 GB/s/core vs ~7 us of PE work hidden under it).
"""

import numpy as np

import concourse.bacc as bacc
import concourse.tile as tile
from concourse import mybir
from concourse.bass_utils import run_bass_kernel_spmd

N_CORES = 8
BATCH = 1024
SEQ = 4096
R = BATCH // N_CORES          # rows per core = 128
P = 128                       # partitions
BLK = 122                     # output columns per block (122 + 6 tap halo = 128)
NBLK = 34                     # ceil(4096 / 122); last block has 70 real columns
HALF = BLK // 2               # 61 even (approx) + 61 odd (detail) cols per block
HCOLS = BLK + NBLK * P        # 122 (band matrix) + 4352 (lhsT tiles)

# chunks of blocks: (first block, n blocks). Each chunk = one input DMA,
# one output DMA; psum groups of <=4 blocks inside. Progressive sizes: small
# first chunk -> PE starts early; small last chunk -> short exposed tail store.
# (verified on HW at rel err 8.3e-08; TimelineSim 16586 ns/core)
CHUNKS = [(0, 2), (2, 5), (7, 9), (16, 9), (25, 6), (31, 3)]

FP32 = mybir.dt.float32

# tuning knobs (see _build_bass); defaults picked via TimelineSim + HW slope
OPTS = {
    "chunks": CHUNKS,
    "alt_copy": True,    # alternate deinterleave copies between DVE and ACT
    "alt_load": True,    # alternate load DMAs between the two HWDGE rings
    "mm_dtype": "f32",   # "f32" | "f32r" (bitcast matmul operands to float32r)
}

_CACHE = {}


def _build_bass(repeat=1, opts=None):
    """Build (once) the single-core Bass/Tile program; all 8 cores run it SPMD.

    repeat > 1 replicates the whole body back-to-back inside one NEFF —
    used only for benchmarking (wall-clock slope vs repeat count isolates
    per-pass HW time from host/tunnel dispatch overhead)."""
    o = dict(OPTS, **(opts or {}))
    chunks = o["chunks"]
    loop_n = o.get("loop_n", 0)  # >0: wrap body in a HW loop (bench only)
    nc = bacc.Bacc(
        "TRN2",
        target_bir_lowering=False,
        debug=False,
        enable_asserts=False,
        num_devices=N_CORES,
    )
    h_t = nc.dram_tensor("h", [P, HCOLS], FP32, kind="ExternalInput")
    out_t = nc.dram_tensor("out", [R, SEQ], FP32, kind="ExternalOutput")
    h_ap = h_t.ap()
    out_ap = out_t.ap()

    with tile.TileContext(nc) as tc:
        with (
            tc.tile_pool(name="hpool", bufs=o.get("hbufs", 4)) as hp,
            tc.tile_pool(name="opool", bufs=o.get("obufs", 4)) as op,
            tc.tile_pool(name="psum", bufs=8, space="PSUM") as psump,
        ):
            # out DRAM viewed as [p, 2 halves, 2048]: half 0 = approx, 1 = detail
            out_v = out_ap.rearrange("p (s m) -> p s m", s=2)

            def mm_ap(ap):
                if o["mm_dtype"] == "f32r":
                    return ap.bitcast(mybir.dt.float32r)
                return ap

            def emit_pass():
                btile = None
                copy_i = 0
                for ci, (b0, nb) in enumerate(chunks):
                    btile, copy_i = emit_chunk(ci, b0, nb, btile, copy_i)

            def emit_chunk(ci, b0, nb, btile, copy_i):
                # chunk 0's DMA also carries the 122-col band matrix so the
                # first matmuls need exactly one DMA wait.
                lead = BLK if b0 == 0 else 0
                dcol0 = BLK + P * b0 - lead
                ht = hp.tile([P, lead + P * nb], FP32, tag="h")
                ld_eng = nc.scalar if (o["alt_load"] and ci % 2) else nc.sync
                ld_eng.dma_start(ht[:], h_ap[:, dcol0 : BLK + P * (b0 + nb)])
                if b0 == 0:
                    btile = ht  # band matrix lives in cols [0:122] of chunk 0

                # number of real output cols this chunk contributes per half
                ceff = min(HALF * (b0 + nb), SEQ // 2) - HALF * b0
                otile = op.tile([P, 2 * ceff], FP32, tag="o")
                o_v = otile[:].rearrange("p (s m) -> p s m", s=2)

                def copy(dst, src):
                    nonlocal copy_i
                    if o["alt_copy"] and copy_i % 2:
                        nc.scalar.copy(dst, src)
                    else:
                        nc.vector.tensor_copy(dst, src)
                    copy_i += 1

                for g0 in range(0, nb, 4):
                    gn = min(4, nb - g0)
                    ps = psump.tile([P, BLK * 4], FP32, tag="ps")
                    for q in range(gn):
                        blk = b0 + g0 + q
                        col = lead + P * (blk - b0) if b0 == 0 else P * (blk - b0)
                        nc.tensor.matmul(
                            ps[:, BLK * q : BLK * (q + 1)],
                            mm_ap(ht[:, col : col + P]),
                            mm_ap(btile[:, 0:BLK]),
                            start=True,
                            stop=True,
                        )
                    # de-interleaving PSUM -> SBUF copy. Full blocks in one
                    # 4D-AP copy; the final 70-wide block separately.
                    nfull = gn if (b0 + g0 + gn) % NBLK else gn - 1
                    loc0 = HALF * g0  # chunk-local col offset of group
                    if nfull:
                        src = ps[:, 0 : BLK * nfull].rearrange(
                            "p (g s t) -> p g s t", s=2, t=HALF
                        )
                        dst = o_v[:, :, loc0 : loc0 + HALF * nfull].rearrange(
                            "p s (g t) -> p g s t", t=HALF
                        )
                        copy(dst, src)
                    if nfull != gn:  # last block: 70 real cols = 35 + 35
                        src = ps[:, BLK * nfull : BLK * (nfull + 1)].rearrange(
                            "p (s t) -> p s t", t=HALF
                        )[:, :, 0:35]
                        dst = o_v[:, :, loc0 + HALF * nfull : loc0 + HALF * nfull + 35]
                        copy(dst, src)

                st_eng = nc.sync if (o["alt_load"] and ci % 2) else nc.scalar
                st_eng.dma_start(
                    out_v[:, :, HALF * b0 : HALF * b0 + ceff], o_v[:, :, :]
                )
                return btile, copy_i

            if loop_n:
                with tc.For_i(0, loop_n, 1):
                    emit_pass()
            else:
                for _ in range(repeat):
                    emit_pass()

    # Note: instructions that end up with >1 sync wait (walrus encodes only
    # one on fp32 LDW+MM pairs etc.) are legalized by bacc's compile() below.
    nc.compile()
    return nc


def _get_nc(repeat=1, opts=None):
    key = ("nc", repeat, repr(sorted((opts or {}).items(), key=str)))
    if key not in _CACHE:
        _CACHE[key] = _build_bass(repeat, opts)
    return _CACHE[key]


def _pack_host(x, bmat):
    """Per-core input tensors: [band matrix | lhsT tiles], where lhsT tile i
    is x_shard.T[122i : 122i+128, :] (circularly padded)."""
    hs = []
    for c in range(N_CORES):
        xs = np.ascontiguousarray(x[R * c : R * (c + 1)].T)  # [4096, 128]
        xtp = np.concatenate([xs, xs[:P]], axis=0)            # circular pad
        H = np.empty((P, HCOLS), dtype=np.float32)
        H[:, 0:BLK] = bmat
        for i in range(NBLK):
            H[:, BLK + P * i : BLK + P * (i + 1)] = xtp[BLK * i : BLK * i + P]
        hs.append(H)
    return hs


def _band_matrix(W):
    """128x122 coefficient block with de-interleaved (evens-first) columns."""
    perm = np.concatenate([np.arange(0, BLK, 2), np.arange(1, BLK, 2)])
    return np.ascontiguousarray(np.asarray(W, dtype=np.float32)[0:P, perm])


def run(x, W, trace=False):
    x = np.ascontiguousarray(np.asarray(x, dtype=np.float32))
    assert x.shape == (BATCH, SEQ), x.shape
    in_maps = [{"h": h} for h in _pack_host(x, _band_matrix(W))]
    res = run_bass_kernel_spmd(
        _get_nc(), in_maps, core_ids=list(range(N_CORES)), trace=trace
    )
    out = np.concatenate([res.results[c]["out"] for c in range(N_CORES)], axis=0)
    return out, res


def kernel(x, W):
    out, _ = run(x, W)
    return out
